# revision 8
# baseline (speedup 1.0000x reference)
"""Trainium2 Bass kernel for nn_BC_5274219839877.

Computes, for b=64, n_v=128, n_q=32, d_v=2048, d_q=1024, K=3072, H=8:
    v_ = relu((v_g/||v_w||) * v @ v_w^T + v_b)        [b, n_v, K]
    q_ = relu((q_g/||q_w||) * q @ q_w^T + q_b)        [b, n_q, K]
    out[b,h,i,j] = sum_k hm[h,k] v_[b,i,k] q_[b,j,k] + h_bias[h]

Sharding: data-parallel over batch across 8 NeuronCores (8 batches/core),
weights replicated. The whole pipeline is fused and k-blocked on-device;
v_/q_ never touch DRAM.

All matmul operands are f32r (measured faster per row than bf16 on this
part), DMA'd directly into f32r SBUF tiles — no staging casts. The bhvq
contraction over k accumulates directly in PSUM banks across all 24
k-blocks (groups opened by zero matmuls, since start=True zeroes the
whole 2KB bank region), so there are no per-block DVE accumulate adds.
"""

import numpy as np

import concourse.bass as bass
import concourse.tile as tile
from concourse import bacc, mybir
from concourse.bass_utils import run_bass_kernel_spmd

F32 = mybir.dt.float32
F32R = mybir.dt.float32r
BF16 = mybir.dt.bfloat16

N_CORES = 8
B = 64
B_LOC = B // N_CORES       # 8 batches per core
NV = 128
NQ = 32
DV = 2048
DQ = 1024
K = 3072
H = 8

KB = 128                   # k-block size (PSUM partition dim)
NKB = K // KB              # 24 k-blocks
TV = DV // 128             # 16 d-tiles (v side)
TQ = DQ // 128             # 8 d-tiles (q side)
MV = B_LOC * NV            # 1024
MQ = B_LOC * NQ            # 256

N_WARM = 90                # 256-row warm-up matmuls (PE p-state ramp)

_CACHE = {}


def _build_program():
    nc = bacc.Bacc("TRN2", target_bir_lowering=False, debug=False,
                   num_devices=N_CORES)

    vt_d = nc.dram_tensor("vt", [TV // 2, 128, 2 * MV], BF16,
                          kind="ExternalInput")
    qt_d = nc.dram_tensor("qt", [TQ // 4, 128, 4 * MQ], F32R,
                          kind="ExternalInput")
    wv_d = nc.dram_tensor("wv", [NKB, 128, TV * KB], F32R,
                          kind="ExternalInput")
    wq_d = nc.dram_tensor("wq", [NKB, 128, TQ * KB], F32R,
                          kind="ExternalInput")
    msb_d = nc.dram_tensor("msb", [128, NKB * H], F32, kind="ExternalInput")
    vb_d = nc.dram_tensor("vb", [128, NKB], F32, kind="ExternalInput")
    qb_d = nc.dram_tensor("qb", [128, NKB], F32, kind="ExternalInput")
    bias_d = nc.dram_tensor("bias", [128, 2 * H * NQ], F32,
                            kind="ExternalInput")
    ssb_d = nc.dram_tensor("ssb", [128, 2], F32, kind="ExternalInput")
    out_d = nc.dram_tensor("out", [128, B_LOC * H * NQ], F32,
                           kind="ExternalOutput")

    relu = mybir.ActivationFunctionType.Relu

    # fixed SBUF allocations (no tile-pool slot recycling)
    msb = nc.alloc_sbuf_tensor("msb_s", [128, NKB * H], F32).ap()
    vb = nc.alloc_sbuf_tensor("vb_s", [128, NKB], F32).ap()
    qb = nc.alloc_sbuf_tensor("qb_s", [128, NKB], F32).ap()
    bias = nc.alloc_sbuf_tensor("bias_s", [128, 2 * H * NQ], F32).ap()
    ssb = nc.alloc_sbuf_tensor("ssb_s", [128, 2], F32).ap()
    vt_big = nc.alloc_sbuf_tensor("vts", [128, TV * MV], F32R).ap()
    qt_big = nc.alloc_sbuf_tensor("qts", [128, TQ * MQ], F32R).ap()
    wv_s = [nc.alloc_sbuf_tensor(f"wvs{i}", [128, TV * KB], F32R).ap()
            for i in range(4)]
    wq_s = [nc.alloc_sbuf_tensor(f"wqs{i}", [128, TQ * KB], F32R).ap()
            for i in range(4)]
    vk = [nc.alloc_sbuf_tensor(f"vk{i}", [128, MV], F32R).ap()
          for i in range(3)]
    qk = [nc.alloc_sbuf_tensor(f"qk{i}", [128, MQ], F32).ap()
          for i in range(2)]
    qx = [nc.alloc_sbuf_tensor(f"qx{i}", [128, H * MQ], F32R).ap()
          for i in range(2)]
    oacc = nc.alloc_sbuf_tensor("oacc", [128, B_LOC * H * NQ], F32).ap()

    vstg = [nc.alloc_sbuf_tensor(f"vstg{i}", [128, 2 * MV], BF16).ap()
            for i in range(2)]
    warm = nc.alloc_sbuf_tensor("warm", [128, 512], BF16).ap()
    psv = [nc.alloc_psum_tensor(f"psv{i}", [128, 512], F32).ap()
           for i in range(2)]
    psq = [nc.alloc_psum_tensor(f"psq{i}", [128, MQ], F32).ap()
           for i in range(2)]
    psa = [nc.alloc_psum_tensor(f"psa{i}", [128, 512], F32).ap()
           for i in range(4)]

    with tile.TileContext(nc) as tc:
        # --- persistent small loads (gpsimd swdge; big drain overlaps) ---
        nc.gpsimd.dma_start(msb, msb_d.ap())
        nc.gpsimd.dma_start(vb, vb_d.ap())
        nc.gpsimd.dma_start(qb, qb_d.ap())
        nc.gpsimd.dma_start(bias, bias_d.ap())
        nc.gpsimd.dma_start(ssb, ssb_d.ap())

        def issue_weights(kb):
            nc.sync.dma_start(wv_s[kb % 4], wv_d[kb])
            nc.scalar.dma_start(wq_s[kb % 4], wq_d[kb])

        # PE pre-warm: dummy matmuls on zeros while the prologue DMAs
        # stream, so the real matmul stream starts at the warm PE clock
        nc.vector.memset(warm, 0.0)
        for i in range(N_WARM):
            nc.tensor.matmul(psq[0][:], warm[:, :128], warm[:, :MQ],
                             start=True, stop=True)
        # open the four stage-3 PSUM accumulation groups with zeros:
        # start=True zeroes the whole 2KB bank, so each bank gets exactly
        # one start and the per-(kb,b) matmuls below only ever accumulate
        for i in range(4):
            nc.tensor.matmul(psa[i][:], warm[:, :128], warm[:],
                             start=True, stop=False, skip_group_check=True)

        # strict DMA priority via per-queue FIFO order: kb0 weights,
        # then vt (shipped as bf16 at half the bytes, upcast to f32r by
        # the otherwise-idle vector/gpsimd engines), then qt, then later
        # weight blocks — so vt lands as early as possible
        issue_weights(0)
        for c in range(TV // 2):
            eng = nc.sync if c % 2 == 0 else nc.scalar
            st = vstg[c % 2]
            eng.dma_start(st, vt_d[c])
            ceng = nc.vector if c % 2 == 0 else nc.gpsimd
            ceng.tensor_copy(vt_big[:, c * 2 * MV:(c + 1) * 2 * MV], st)
        for c in range(TQ // 4):
            eng = nc.sync if c % 2 == 0 else nc.scalar
            eng.dma_start(qt_big[:, c * 4 * MQ:(c + 1) * 4 * MQ], qt_d[c])
        issue_weights(1)
        issue_weights(2)

        def stage3(kb):
            # psa[b//2][:, (b%2)*256:] += vk[:, b].T @ Qx[:, b, :, :]
            # accumulated in PSUM across all NKB k-blocks
            vkb = vk[kb % 3]
            qxb = qx[kb % 2]
            for b_ in range(B_LOC):
                nc.tensor.matmul(
                    psa[b_ // 2][:, (b_ % 2) * H * NQ:(b_ % 2 + 1) * H * NQ],
                    vkb[:, b_ * NV:(b_ + 1) * NV],
                    qxb[:, b_ * H * NQ:(b_ + 1) * H * NQ],
                    start=False, stop=(kb == NKB - 1),
                    skip_group_check=True)

        # --- k-blocked fused pipeline ---
        for kb in range(NKB):
            # prefetch weights for kb+3: with 4-deep buffers this has no
            # WAR gate on the current iteration, so the DMA starts early
            if kb + 3 < NKB:
                issue_weights(kb + 3)
            wvb = wv_s[kb % 4]
            wqb = wq_s[kb % 4]

            # stage 1 (v): vk[k, m] = relu(s_v * (v @ v_w^T)^T + v_b)
            vkb = vk[kb % 3]
            for mc in range(MV // 512):
                ps = psv[mc]
                for t in range(TV):
                    nc.tensor.matmul(
                        ps[:],
                        wvb[:, t * KB:(t + 1) * KB],
                        vt_big[:, t * MV + mc * 512:t * MV + (mc + 1) * 512],
                        start=(t == 0), stop=(t == TV - 1))
                nc.scalar.activation(
                    vkb[:, mc * 512:(mc + 1) * 512], ps[:], relu,
                    bias=vb[:, kb:kb + 1], scale=ssb[:, 0:1])

            # stage 1 (q): qk[k, m2]
            qp = psq[kb % 2]
            for t in range(TQ):
                nc.tensor.matmul(
                    qp[:],
                    wqb[:, t * KB:(t + 1) * KB],
                    qt_big[:, t * MQ:(t + 1) * MQ],
                    start=(t == 0), stop=(t == TQ - 1))
            qkb = qk[kb % 2]
            nc.scalar.activation(qkb[:], qp[:], relu,
                                 bias=qb[:, kb:kb + 1], scale=ssb[:, 1:2])

            # stage 2: Qx[k, b*(H*NQ) + h*NQ + j] = hm[h, k] * qk[k, (b,j)]
            qxb = qx[kb % 2]
            qx4 = qxb.rearrange("p (b h j) -> p b h j", b=B_LOC, h=H)
            qk3 = qkb.rearrange("p (b j) -> p b j", b=B_LOC)
            for h in range(H):
                nc.vector.tensor_scalar_mul(
                    qx4[:, :, h, :], qk3[:, :, :],
                    msb[:, kb * H + h:kb * H + h + 1])

            # stage 3, pipelined one k-block behind so its matmuls never
            # head-of-line block ready stage-1 matmuls on the PE stream
            if kb >= 1:
                stage3(kb - 1)

        stage3(NKB - 1)

        # epilogue: bias-add out of PSUM (vector only: gpsimd can't read
        # PSUM), each bank's DMA issued as soon as its add lands
        for i in range(4):
            nc.vector.tensor_add(
                oacc[:, i * 512:(i + 1) * 512], psa[i][:], bias[:])
            eng = nc.sync if i % 2 == 0 else nc.scalar
            eng.dma_start(out_d.ap()[:, i * 512:(i + 1) * 512],
                          oacc[:, i * 512:(i + 1) * 512])

    nc.compile()
    return nc


def _prep_host(inputs):
    v = np.ascontiguousarray(np.asarray(inputs["v"], dtype=np.float32))
    q = np.ascontiguousarray(np.asarray(inputs["q"], dtype=np.float32))
    v_w = np.asarray(inputs["v_w"], dtype=np.float32)
    q_w = np.asarray(inputs["q_w"], dtype=np.float32)
    v_g = float(np.asarray(inputs["v_g"], dtype=np.float32))
    q_g = float(np.asarray(inputs["q_g"], dtype=np.float32))
    v_b = np.asarray(inputs["v_b"], dtype=np.float32)
    q_b = np.asarray(inputs["q_b"], dtype=np.float32)
    h_mat = np.asarray(inputs["h_mat"], dtype=np.float32)
    h_bias = np.asarray(inputs["h_bias"], dtype=np.float32)

    s_v = v_g / float(np.linalg.norm(v_w.astype(np.float32)))
    s_q = q_g / float(np.linalg.norm(q_w.astype(np.float32)))

    wv_r = np.ascontiguousarray(
        v_w.reshape(NKB, KB, TV, 128).transpose(0, 3, 2, 1)
        .reshape(NKB, 128, TV * KB))
    wq_r = np.ascontiguousarray(
        q_w.reshape(NKB, KB, TQ, 128).transpose(0, 3, 2, 1)
        .reshape(NKB, 128, TQ * KB))
    hm = h_mat[0, :, 0, :]                       # [H, K]
    msb = np.ascontiguousarray(
        hm.T.reshape(NKB, 128, H).transpose(1, 0, 2).reshape(128, NKB * H))
    vb_r = np.ascontiguousarray(v_b.reshape(NKB, 128).T)
    qb_r = np.ascontiguousarray(q_b.reshape(NKB, 128).T)
    hb = h_bias[0, :, 0, 0]                      # [H]
    bias = np.ascontiguousarray(
        np.broadcast_to(np.tile(np.repeat(hb, NQ), 2)[None, :],
                        (128, 2 * H * NQ)))
    ssb = np.ascontiguousarray(
        np.broadcast_to(np.array([s_v, s_q], dtype=np.float32)[None, :],
                        (128, 2)))

    in_maps = []
    for c in range(N_CORES):
        vc = v[c * B_LOC:(c + 1) * B_LOC]        # [B_LOC, NV, DV]
        qc = q[c * B_LOC:(c + 1) * B_LOC]        # [B_LOC, NQ, DQ]
        bf16 = mybir.dt.np(BF16)
        vt_c = np.ascontiguousarray(
            vc.reshape(B_LOC, NV, TV, 128).transpose(2, 3, 0, 1)
            .reshape(TV // 2, 2, 128, MV).transpose(0, 2, 1, 3)
            .reshape(TV // 2, 128, 2 * MV).astype(bf16))
        qt_c = np.ascontiguousarray(
            qc.reshape(B_LOC, NQ, TQ, 128).transpose(2, 3, 0, 1)
            .reshape(TQ // 4, 4, 128, MQ).transpose(0, 2, 1, 3)
            .reshape(TQ // 4, 128, 4 * MQ))
        in_maps.append({
            "vt": vt_c, "qt": qt_c, "wv": wv_r, "wq": wq_r,
            "msb": msb, "vb": vb_r, "qb": qb_r, "bias": bias, "ssb": ssb,
        })
    return in_maps


def _run(inputs, trace=False):
    if "nc" not in _CACHE:
        _CACHE["nc"] = _build_program()
    nc = _CACHE["nc"]
    in_maps = _prep_host(inputs)
    res = run_bass_kernel_spmd(nc, in_maps, list(range(N_CORES)), trace=trace)
    out = np.empty((B, H, NV, NQ), dtype=np.float32)
    for c in range(N_CORES):
        oc = res.results[c]["out"]               # [128, B_LOC*H*NQ]
        out[c * B_LOC:(c + 1) * B_LOC] = (
            oc.reshape(NV, B_LOC, H, NQ).transpose(1, 2, 0, 3))
    return out, res


def kernel(**inputs):
    return _run(inputs)[0]


# revision 10
# speedup vs baseline: 1.0447x; 1.0447x over previous
"""Trainium2 Bass kernel for nn_BC_5274219839877.

Computes, for b=64, n_v=128, n_q=32, d_v=2048, d_q=1024, K=3072, H=8:
    v_ = relu((v_g/||v_w||) * v @ v_w^T + v_b)        [b, n_v, K]
    q_ = relu((q_g/||q_w||) * q @ q_w^T + q_b)        [b, n_q, K]
    out[b,h,i,j] = sum_k hm[h,k] v_[b,i,k] q_[b,j,k] + h_bias[h]

Sharding: data-parallel over batch across 8 NeuronCores (8 batches/core),
weights replicated. The whole pipeline is fused and k-blocked on-device;
v_/q_ never touch DRAM.

All matmul operands are f32r (measured faster per row than bf16 on this
part), DMA'd directly into f32r SBUF tiles — no staging casts. The bhvq
contraction over k accumulates directly in PSUM banks across all 24
k-blocks (groups opened by zero matmuls, since start=True zeroes the
whole 2KB bank region), so there are no per-block DVE accumulate adds.
"""

import numpy as np

import concourse.bass as bass
import concourse.tile as tile
from concourse import bacc, mybir
from concourse.bass_utils import run_bass_kernel_spmd

F32 = mybir.dt.float32
F32R = mybir.dt.float32r
BF16 = mybir.dt.bfloat16

N_CORES = 8
B = 64
B_LOC = B // N_CORES       # 8 batches per core
NV = 128
NQ = 32
DV = 2048
DQ = 1024
K = 3072
H = 8

KB = 128                   # k-block size (PSUM partition dim)
NKB = K // KB              # 24 k-blocks
TV = DV // 128             # 16 d-tiles (v side)
TQ = DQ // 128             # 8 d-tiles (q side)
MV = B_LOC * NV            # 1024
MQ = B_LOC * NQ            # 256

N_WARM = 70                # 256-row warm-up matmuls (PE p-state ramp)

_CACHE = {}


def _build_program():
    nc = bacc.Bacc("TRN2", target_bir_lowering=False, debug=False,
                   num_devices=N_CORES)

    vt_d = nc.dram_tensor("vt", [TV // 2, 128, 2 * MV], F32R,
                          kind="ExternalInput")
    qt_d = nc.dram_tensor("qt", [TQ // 4, 128, 4 * MQ], F32R,
                          kind="ExternalInput")
    wv_d = nc.dram_tensor("wv", [NKB, 128, TV * KB], F32R,
                          kind="ExternalInput")
    wq_d = nc.dram_tensor("wq", [NKB, 128, TQ * KB], F32R,
                          kind="ExternalInput")
    msb_d = nc.dram_tensor("msb", [128, NKB * H], F32, kind="ExternalInput")
    vb_d = nc.dram_tensor("vb", [128, NKB], F32, kind="ExternalInput")
    qb_d = nc.dram_tensor("qb", [128, NKB], F32, kind="ExternalInput")
    bias_d = nc.dram_tensor("bias", [1, 2 * H * NQ], F32R,
                            kind="ExternalInput")
    ssb_d = nc.dram_tensor("ssb", [128, 2], F32, kind="ExternalInput")
    out_d = nc.dram_tensor("out", [128, B_LOC * H * NQ], F32,
                           kind="ExternalOutput")

    relu = mybir.ActivationFunctionType.Relu

    # fixed SBUF allocations (no tile-pool slot recycling)
    msb = nc.alloc_sbuf_tensor("msb_s", [128, NKB * H], F32).ap()
    vb = nc.alloc_sbuf_tensor("vb_s", [128, NKB], F32).ap()
    qb = nc.alloc_sbuf_tensor("qb_s", [128, NKB], F32).ap()
    bias = nc.alloc_sbuf_tensor("bias_s", [1, 2 * H * NQ], F32R).ap()
    ssb = nc.alloc_sbuf_tensor("ssb_s", [128, 2], F32).ap()
    vt_big = nc.alloc_sbuf_tensor("vts", [128, TV * MV], F32R).ap()
    qt_big = nc.alloc_sbuf_tensor("qts", [128, TQ * MQ], F32R).ap()
    wv_s = [nc.alloc_sbuf_tensor(f"wvs{i}", [128, TV * KB], F32R).ap()
            for i in range(4)]
    wq_s = [nc.alloc_sbuf_tensor(f"wqs{i}", [128, TQ * KB], F32R).ap()
            for i in range(4)]
    vk = [nc.alloc_sbuf_tensor(f"vk{i}", [128, MV], F32R).ap()
          for i in range(3)]
    qk = [nc.alloc_sbuf_tensor(f"qk{i}", [128, MQ], F32).ap()
          for i in range(2)]
    qx = [nc.alloc_sbuf_tensor(f"qx{i}", [128, H * MQ], F32R).ap()
          for i in range(2)]
    oacc = nc.alloc_sbuf_tensor("oacc", [128, B_LOC * H * NQ], F32).ap()

    ones = nc.alloc_sbuf_tensor("ones", [1, 128], F32).ap()
    warm = nc.alloc_sbuf_tensor("warm", [128, 512], BF16).ap()
    psv = [nc.alloc_psum_tensor(f"psv{i}", [128, 512], F32).ap()
           for i in range(2)]
    psq = [nc.alloc_psum_tensor(f"psq{i}", [128, MQ], F32).ap()
           for i in range(2)]
    psa = [nc.alloc_psum_tensor(f"psa{i}", [128, 512], F32).ap()
           for i in range(4)]

    with tile.TileContext(nc) as tc:
        # --- persistent small loads (gpsimd swdge; big drain overlaps) ---
        nc.gpsimd.dma_start(msb, msb_d.ap())
        nc.gpsimd.dma_start(vb, vb_d.ap())
        nc.gpsimd.dma_start(qb, qb_d.ap())
        nc.gpsimd.dma_start(bias, bias_d.ap())
        nc.gpsimd.dma_start(ssb, ssb_d.ap())

        def issue_weights(kb):
            nc.sync.dma_start(wv_s[kb % 4], wv_d[kb])
            nc.scalar.dma_start(wq_s[kb % 4], wq_d[kb])

        # PE pre-warm: dummy matmuls on zeros while the prologue DMAs
        # stream, so the real matmul stream starts at the warm PE clock
        nc.vector.memset(warm, 0.0)
        for i in range(N_WARM):
            nc.tensor.matmul(psq[0][:], warm[:, :128], warm[:, :MQ],
                             start=True, stop=True)
        # open the four stage-3 PSUM accumulation groups with zeros:
        # start=True zeroes the whole 2KB bank, so each bank gets exactly
        # one start and the per-(kb,b) matmuls below only ever accumulate
        for i in range(4):
            nc.tensor.matmul(psa[i][:], warm[:, :128], warm[:],
                             start=True, stop=False, skip_group_check=True)

        # strict DMA priority via per-queue FIFO order: kb0 weights,
        # then vt balanced across both hwdge queues, then qt, then later
        # weight blocks — so vt lands as early as possible
        nc.vector.memset(ones, 1.0)
        issue_weights(0)
        for c in range(TV // 2):
            eng = nc.sync if c % 2 == 0 else nc.scalar
            eng.dma_start(vt_big[:, c * 2 * MV:(c + 1) * 2 * MV], vt_d[c])
        for c in range(TQ // 4):
            eng = nc.sync if c % 2 == 0 else nc.scalar
            eng.dma_start(qt_big[:, c * 4 * MQ:(c + 1) * 4 * MQ], qt_d[c])
        issue_weights(1)
        issue_weights(2)

        def stage3(kb):
            # psa[b//2][:, (b%2)*256:] += vk[:, b].T @ Qx[:, b, :, :]
            # accumulated in PSUM across all NKB k-blocks
            vkb = vk[kb % 3]
            qxb = qx[kb % 2]
            for b_ in range(B_LOC):
                nc.tensor.matmul(
                    psa[b_ // 2][:, (b_ % 2) * H * NQ:(b_ % 2 + 1) * H * NQ],
                    vkb[:, b_ * NV:(b_ + 1) * NV],
                    qxb[:, b_ * H * NQ:(b_ + 1) * H * NQ],
                    start=False, stop=False,
                    skip_group_check=True)

        # --- k-blocked fused pipeline ---
        for kb in range(NKB):
            # prefetch weights for kb+3: with 4-deep buffers this has no
            # WAR gate on the current iteration, so the DMA starts early
            if kb + 3 < NKB:
                issue_weights(kb + 3)
            wvb = wv_s[kb % 4]
            wqb = wq_s[kb % 4]

            # stage 1 (v): vk[k, m] = relu(s_v * (v @ v_w^T)^T + v_b)
            vkb = vk[kb % 3]
            for mc in range(MV // 512):
                ps = psv[mc]
                for t in range(TV):
                    nc.tensor.matmul(
                        ps[:],
                        wvb[:, t * KB:(t + 1) * KB],
                        vt_big[:, t * MV + mc * 512:t * MV + (mc + 1) * 512],
                        start=(t == 0), stop=(t == TV - 1))
                nc.scalar.activation(
                    vkb[:, mc * 512:(mc + 1) * 512], ps[:], relu,
                    bias=vb[:, kb:kb + 1], scale=ssb[:, 0:1])

            # stage 1 (q): qk[k, m2]
            qp = psq[kb % 2]
            for t in range(TQ):
                nc.tensor.matmul(
                    qp[:],
                    wqb[:, t * KB:(t + 1) * KB],
                    qt_big[:, t * MQ:(t + 1) * MQ],
                    start=(t == 0), stop=(t == TQ - 1))
            qkb = qk[kb % 2]
            nc.scalar.activation(qkb[:], qp[:], relu,
                                 bias=qb[:, kb:kb + 1], scale=ssb[:, 1:2])

            # stage 2: Qx[k, b*(H*NQ) + h*NQ + j] = hm[h, k] * qk[k, (b,j)]
            qxb = qx[kb % 2]
            qx4 = qxb.rearrange("p (b h j) -> p b h j", b=B_LOC, h=H)
            qk3 = qkb.rearrange("p (b j) -> p b j", b=B_LOC)
            for h in range(H):
                nc.vector.tensor_scalar_mul(
                    qx4[:, :, h, :], qk3[:, :, :],
                    msb[:, kb * H + h:kb * H + h + 1])

            # stage 3, pipelined one k-block behind so its matmuls never
            # head-of-line block ready stage-1 matmuls on the PE stream
            if kb >= 1:
                stage3(kb - 1)

        stage3(NKB - 1)

        # epilogue: h_bias folded in exactly via a rank-1 PE accumulate
        # per bank, then PSUM read-out split across vector+scalar, each
        # bank's DMA issued as soon as its copy lands
        cp = mybir.ActivationFunctionType.Copy
        for i in range(4):
            nc.tensor.matmul(psa[i][:], ones.bitcast(F32R)[:], bias[:],
                             start=False, stop=True, skip_group_check=True)
            if i % 2 == 0:
                nc.vector.tensor_copy(oacc[:, i * 512:(i + 1) * 512],
                                      psa[i][:])
            else:
                nc.scalar.activation(oacc[:, i * 512:(i + 1) * 512],
                                     psa[i][:], cp)
            eng = nc.sync if i % 2 == 0 else nc.scalar
            eng.dma_start(out_d.ap()[:, i * 512:(i + 1) * 512],
                          oacc[:, i * 512:(i + 1) * 512])

    nc.compile()
    return nc


def _prep_host(inputs):
    v = np.ascontiguousarray(np.asarray(inputs["v"], dtype=np.float32))
    q = np.ascontiguousarray(np.asarray(inputs["q"], dtype=np.float32))
    v_w = np.asarray(inputs["v_w"], dtype=np.float32)
    q_w = np.asarray(inputs["q_w"], dtype=np.float32)
    v_g = float(np.asarray(inputs["v_g"], dtype=np.float32))
    q_g = float(np.asarray(inputs["q_g"], dtype=np.float32))
    v_b = np.asarray(inputs["v_b"], dtype=np.float32)
    q_b = np.asarray(inputs["q_b"], dtype=np.float32)
    h_mat = np.asarray(inputs["h_mat"], dtype=np.float32)
    h_bias = np.asarray(inputs["h_bias"], dtype=np.float32)

    s_v = v_g / float(np.linalg.norm(v_w.astype(np.float32)))
    s_q = q_g / float(np.linalg.norm(q_w.astype(np.float32)))

    wv_r = np.ascontiguousarray(
        v_w.reshape(NKB, KB, TV, 128).transpose(0, 3, 2, 1)
        .reshape(NKB, 128, TV * KB))
    wq_r = np.ascontiguousarray(
        q_w.reshape(NKB, KB, TQ, 128).transpose(0, 3, 2, 1)
        .reshape(NKB, 128, TQ * KB))
    hm = h_mat[0, :, 0, :]                       # [H, K]
    msb = np.ascontiguousarray(
        hm.T.reshape(NKB, 128, H).transpose(1, 0, 2).reshape(128, NKB * H))
    vb_r = np.ascontiguousarray(v_b.reshape(NKB, 128).T)
    qb_r = np.ascontiguousarray(q_b.reshape(NKB, 128).T)
    hb = h_bias[0, :, 0, 0]                      # [H]
    bias = np.ascontiguousarray(
        np.tile(np.repeat(hb, NQ), 2)[None, :].astype(np.float32))
    ssb = np.ascontiguousarray(
        np.broadcast_to(np.array([s_v, s_q], dtype=np.float32)[None, :],
                        (128, 2)))

    in_maps = []
    for c in range(N_CORES):
        vc = v[c * B_LOC:(c + 1) * B_LOC]        # [B_LOC, NV, DV]
        qc = q[c * B_LOC:(c + 1) * B_LOC]        # [B_LOC, NQ, DQ]
        vt_c = np.ascontiguousarray(
            vc.reshape(B_LOC, NV, TV, 128).transpose(2, 3, 0, 1)
            .reshape(TV // 2, 2, 128, MV).transpose(0, 2, 1, 3)
            .reshape(TV // 2, 128, 2 * MV))
        qt_c = np.ascontiguousarray(
            qc.reshape(B_LOC, NQ, TQ, 128).transpose(2, 3, 0, 1)
            .reshape(TQ // 4, 4, 128, MQ).transpose(0, 2, 1, 3)
            .reshape(TQ // 4, 128, 4 * MQ))
        in_maps.append({
            "vt": vt_c, "qt": qt_c, "wv": wv_r, "wq": wq_r,
            "msb": msb, "vb": vb_r, "qb": qb_r, "bias": bias, "ssb": ssb,
        })
    return in_maps


def _run(inputs, trace=False):
    if "nc" not in _CACHE:
        _CACHE["nc"] = _build_program()
    nc = _CACHE["nc"]
    in_maps = _prep_host(inputs)
    res = run_bass_kernel_spmd(nc, in_maps, list(range(N_CORES)), trace=trace)
    out = np.empty((B, H, NV, NQ), dtype=np.float32)
    for c in range(N_CORES):
        oc = res.results[c]["out"]               # [128, B_LOC*H*NQ]
        out[c * B_LOC:(c + 1) * B_LOC] = (
            oc.reshape(NV, B_LOC, H, NQ).transpose(1, 2, 0, 3))
    return out, res


def kernel(**inputs):
    return _run(inputs)[0]


# revision 11
# speedup vs baseline: 1.0481x; 1.0033x over previous
"""Trainium2 Bass kernel for nn_BC_5274219839877.

Computes, for b=64, n_v=128, n_q=32, d_v=2048, d_q=1024, K=3072, H=8:
    v_ = relu((v_g/||v_w||) * v @ v_w^T + v_b)        [b, n_v, K]
    q_ = relu((q_g/||q_w||) * q @ q_w^T + q_b)        [b, n_q, K]
    out[b,h,i,j] = sum_k hm[h,k] v_[b,i,k] q_[b,j,k] + h_bias[h]

Sharding: data-parallel over batch across 8 NeuronCores (8 batches/core),
weights replicated. The whole pipeline is fused and k-blocked on-device;
v_/q_ never touch DRAM.

All matmul operands are f32r (measured faster per row than bf16 on this
part), DMA'd directly into f32r SBUF tiles — no staging casts. The bhvq
contraction over k accumulates directly in PSUM banks across all 24
k-blocks (groups opened by zero matmuls, since start=True zeroes the
whole 2KB bank region), so there are no per-block DVE accumulate adds.
"""

import numpy as np

import concourse.bass as bass
import concourse.tile as tile
from concourse import bacc, mybir
from concourse.bass_utils import run_bass_kernel_spmd

F32 = mybir.dt.float32
F32R = mybir.dt.float32r
BF16 = mybir.dt.bfloat16

N_CORES = 8
B = 64
B_LOC = B // N_CORES       # 8 batches per core
NV = 128
NQ = 32
DV = 2048
DQ = 1024
K = 3072
H = 8

KB = 128                   # k-block size (PSUM partition dim)
NKB = K // KB              # 24 k-blocks
TV = DV // 128             # 16 d-tiles (v side)
TQ = DQ // 128             # 8 d-tiles (q side)
MV = B_LOC * NV            # 1024
MQ = B_LOC * NQ            # 256

N_WARM = 30                # 256-row warm-up matmuls (PE p-state ramp)

_CACHE = {}


def _build_program():
    nc = bacc.Bacc("TRN2", target_bir_lowering=False, debug=False,
                   num_devices=N_CORES)

    vt_d = nc.dram_tensor("vt", [TV // 2, 128, 2 * MV], BF16,
                          kind="ExternalInput")
    qt_d = nc.dram_tensor("qt", [TQ // 4, 128, 4 * MQ], BF16,
                          kind="ExternalInput")
    wv_d = nc.dram_tensor("wv", [NKB, 128, TV * KB], BF16,
                          kind="ExternalInput")
    wq_d = nc.dram_tensor("wq", [NKB, 128, TQ * KB], BF16,
                          kind="ExternalInput")
    msb_d = nc.dram_tensor("msb", [128, NKB * H], F32, kind="ExternalInput")
    vb_d = nc.dram_tensor("vb", [128, NKB], F32, kind="ExternalInput")
    qb_d = nc.dram_tensor("qb", [128, NKB], F32, kind="ExternalInput")
    bias_d = nc.dram_tensor("bias", [1, 2 * H * NQ], F32R,
                            kind="ExternalInput")
    ssb_d = nc.dram_tensor("ssb", [128, 2], F32, kind="ExternalInput")
    out_d = nc.dram_tensor("out", [128, B_LOC * H * NQ], F32,
                           kind="ExternalOutput")

    relu = mybir.ActivationFunctionType.Relu

    # fixed SBUF allocations (no tile-pool slot recycling)
    msb = nc.alloc_sbuf_tensor("msb_s", [128, NKB * H], F32).ap()
    vb = nc.alloc_sbuf_tensor("vb_s", [128, NKB], F32).ap()
    qb = nc.alloc_sbuf_tensor("qb_s", [128, NKB], F32).ap()
    bias = nc.alloc_sbuf_tensor("bias_s", [1, 2 * H * NQ], F32R).ap()
    ssb = nc.alloc_sbuf_tensor("ssb_s", [128, 2], F32).ap()
    vt_big = nc.alloc_sbuf_tensor("vts", [128, TV * MV], F32R).ap()
    qt_big = nc.alloc_sbuf_tensor("qts", [128, TQ * MQ], F32R).ap()
    wv_g = [nc.alloc_sbuf_tensor(f"wvg{i}", [128, TV * KB], BF16).ap()
            for i in range(2)]
    wq_g = [nc.alloc_sbuf_tensor(f"wqg{i}", [128, TQ * KB], BF16).ap()
            for i in range(2)]
    vt_g = [nc.alloc_sbuf_tensor(f"vtg{i}", [128, 2 * MV], BF16).ap()
            for i in range(2)]
    wv_s = [nc.alloc_sbuf_tensor(f"wvs{i}", [128, TV * KB], F32R).ap()
            for i in range(4)]
    wq_s = [nc.alloc_sbuf_tensor(f"wqs{i}", [128, TQ * KB], F32R).ap()
            for i in range(4)]
    vk = [nc.alloc_sbuf_tensor(f"vk{i}", [128, MV], F32R).ap()
          for i in range(3)]
    qk = [nc.alloc_sbuf_tensor(f"qk{i}", [128, MQ], F32).ap()
          for i in range(2)]
    qx = [nc.alloc_sbuf_tensor(f"qx{i}", [128, H * MQ], F32R).ap()
          for i in range(2)]
    oacc = nc.alloc_sbuf_tensor("oacc", [128, B_LOC * H * NQ], F32).ap()

    ones = nc.alloc_sbuf_tensor("ones", [1, 128], F32).ap()
    warm = nc.alloc_sbuf_tensor("warm", [128, 512], BF16).ap()
    psv = [nc.alloc_psum_tensor(f"psv{i}", [128, 512], F32).ap()
           for i in range(2)]
    psq = [nc.alloc_psum_tensor(f"psq{i}", [128, MQ], F32).ap()
           for i in range(2)]
    psa = [nc.alloc_psum_tensor(f"psa{i}", [128, 512], F32).ap()
           for i in range(4)]

    with tile.TileContext(nc) as tc:
        # --- persistent small loads (gpsimd swdge; big drain overlaps) ---
        nc.gpsimd.dma_start(msb, msb_d.ap())
        nc.gpsimd.dma_start(vb, vb_d.ap())
        nc.gpsimd.dma_start(qb, qb_d.ap())
        nc.gpsimd.dma_start(bias, bias_d.ap())
        nc.gpsimd.dma_start(ssb, ssb_d.ap())

        def issue_weights(kb):
            g = wv_g[kb % 2]
            nc.sync.dma_start(g, wv_d[kb])
            nc.vector.tensor_copy(wv_s[kb % 4], g)
            g2 = wq_g[kb % 2]
            nc.scalar.dma_start(g2, wq_d[kb])
            nc.vector.tensor_copy(wq_s[kb % 4], g2)

        # PE pre-warm: dummy matmuls on zeros while the prologue DMAs
        # stream, so the real matmul stream starts at the warm PE clock
        nc.vector.memset(warm, 0.0)
        for i in range(N_WARM):
            nc.tensor.matmul(psq[0][:], warm[:, :128], warm[:, :MQ],
                             start=True, stop=True)
        # open the four stage-3 PSUM accumulation groups with zeros:
        # start=True zeroes the whole 2KB bank, so each bank gets exactly
        # one start and the per-(kb,b) matmuls below only ever accumulate
        for i in range(4):
            nc.tensor.matmul(psa[i][:], warm[:, :128], warm[:],
                             start=True, stop=False, skip_group_check=True)

        # strict DMA priority via per-queue FIFO order: kb0 weights,
        # then vt (all big inputs ship as bf16 at half the bytes and are
        # upcast to f32r by the otherwise-idle vector engine), with kb1
        # weights slotted before the last vt chunks, then qt, then more
        # weights — so vt and the early weight blocks land early
        nc.vector.memset(ones, 1.0)
        issue_weights(0)

        def vt_chunk(c):
            eng = nc.sync if c % 2 == 0 else nc.scalar
            g = vt_g[c % 2]
            eng.dma_start(g, vt_d[c])
            nc.vector.tensor_copy(
                vt_big[:, c * 2 * MV:(c + 1) * 2 * MV], g)

        for c in range(6):
            vt_chunk(c)
        issue_weights(1)
        vt_chunk(6)
        vt_chunk(7)
        for c in range(TQ // 4):
            eng = nc.sync if c % 2 == 0 else nc.scalar
            g = vt_g[c % 2]
            eng.dma_start(g[:, :4 * MQ], qt_d[c])
            nc.vector.tensor_copy(
                qt_big[:, c * 4 * MQ:(c + 1) * 4 * MQ], g[:, :4 * MQ])
        issue_weights(2)

        def stage3(kb):
            # psa[b//2][:, (b%2)*256:] += vk[:, b].T @ Qx[:, b, :, :]
            # accumulated in PSUM across all NKB k-blocks
            vkb = vk[kb % 3]
            qxb = qx[kb % 2]
            for b_ in range(B_LOC):
                nc.tensor.matmul(
                    psa[b_ // 2][:, (b_ % 2) * H * NQ:(b_ % 2 + 1) * H * NQ],
                    vkb[:, b_ * NV:(b_ + 1) * NV],
                    qxb[:, b_ * H * NQ:(b_ + 1) * H * NQ],
                    start=False, stop=False,
                    skip_group_check=True)

        # --- k-blocked fused pipeline ---
        for kb in range(NKB):
            wvb = wv_s[kb % 4]
            wqb = wq_s[kb % 4]

            # stage 1 (v): vk[k, m] = relu(s_v * (v @ v_w^T)^T + v_b)
            vkb = vk[kb % 3]
            for mc in range(MV // 512):
                ps = psv[mc]
                for t in range(TV):
                    nc.tensor.matmul(
                        ps[:],
                        wvb[:, t * KB:(t + 1) * KB],
                        vt_big[:, t * MV + mc * 512:t * MV + (mc + 1) * 512],
                        start=(t == 0), stop=(t == TV - 1))
                nc.scalar.activation(
                    vkb[:, mc * 512:(mc + 1) * 512], ps[:], relu,
                    bias=vb[:, kb:kb + 1], scale=ssb[:, 0:1])

            # stage 1 (q): qk[k, m2]
            qp = psq[kb % 2]
            for t in range(TQ):
                nc.tensor.matmul(
                    qp[:],
                    wqb[:, t * KB:(t + 1) * KB],
                    qt_big[:, t * MQ:(t + 1) * MQ],
                    start=(t == 0), stop=(t == TQ - 1))
            qkb = qk[kb % 2]
            nc.scalar.activation(qkb[:], qp[:], relu,
                                 bias=qb[:, kb:kb + 1], scale=ssb[:, 1:2])

            # stage 2: Qx[k, b*(H*NQ) + h*NQ + j] = hm[h, k] * qk[k, (b,j)]
            qxb = qx[kb % 2]
            qx4 = qxb.rearrange("p (b h j) -> p b h j", b=B_LOC, h=H)
            qk3 = qkb.rearrange("p (b j) -> p b j", b=B_LOC)
            for h in range(H):
                nc.vector.tensor_scalar_mul(
                    qx4[:, :, h, :], qk3[:, :, :],
                    msb[:, kb * H + h:kb * H + h + 1])

            # stage 3, pipelined one k-block behind so its matmuls never
            # head-of-line block ready stage-1 matmuls on the PE stream
            if kb >= 1:
                stage3(kb - 1)

            # prefetch weights for kb+3: 4-deep f32r buffers mean the
            # cast target was last read at kb-1, so no WAR gate; 2-deep
            # bf16 staging is safe because the cast runs within this kb
            if kb + 3 < NKB:
                issue_weights(kb + 3)

        stage3(NKB - 1)

        # epilogue: h_bias folded in exactly via a rank-1 PE accumulate
        # per bank, then PSUM read-out split across vector+scalar, each
        # bank's DMA issued as soon as its copy lands
        cp = mybir.ActivationFunctionType.Copy
        for i in range(4):
            nc.tensor.matmul(psa[i][:], ones.bitcast(F32R)[:], bias[:],
                             start=False, stop=True, skip_group_check=True)
            if i % 2 == 0:
                nc.vector.tensor_copy(oacc[:, i * 512:(i + 1) * 512],
                                      psa[i][:])
            else:
                nc.scalar.activation(oacc[:, i * 512:(i + 1) * 512],
                                     psa[i][:], cp)
            eng = nc.sync if i % 2 == 0 else nc.scalar
            eng.dma_start(out_d.ap()[:, i * 512:(i + 1) * 512],
                          oacc[:, i * 512:(i + 1) * 512])

    nc.compile()
    return nc


def _prep_host(inputs):
    v = np.ascontiguousarray(np.asarray(inputs["v"], dtype=np.float32))
    q = np.ascontiguousarray(np.asarray(inputs["q"], dtype=np.float32))
    v_w = np.asarray(inputs["v_w"], dtype=np.float32)
    q_w = np.asarray(inputs["q_w"], dtype=np.float32)
    v_g = float(np.asarray(inputs["v_g"], dtype=np.float32))
    q_g = float(np.asarray(inputs["q_g"], dtype=np.float32))
    v_b = np.asarray(inputs["v_b"], dtype=np.float32)
    q_b = np.asarray(inputs["q_b"], dtype=np.float32)
    h_mat = np.asarray(inputs["h_mat"], dtype=np.float32)
    h_bias = np.asarray(inputs["h_bias"], dtype=np.float32)

    s_v = v_g / float(np.linalg.norm(v_w.astype(np.float32)))
    s_q = q_g / float(np.linalg.norm(q_w.astype(np.float32)))

    bf16 = mybir.dt.np(BF16)
    wv_r = np.ascontiguousarray(
        v_w.reshape(NKB, KB, TV, 128).transpose(0, 3, 2, 1)
        .reshape(NKB, 128, TV * KB).astype(bf16))
    wq_r = np.ascontiguousarray(
        q_w.reshape(NKB, KB, TQ, 128).transpose(0, 3, 2, 1)
        .reshape(NKB, 128, TQ * KB).astype(bf16))
    hm = h_mat[0, :, 0, :]                       # [H, K]
    msb = np.ascontiguousarray(
        hm.T.reshape(NKB, 128, H).transpose(1, 0, 2).reshape(128, NKB * H))
    vb_r = np.ascontiguousarray(v_b.reshape(NKB, 128).T)
    qb_r = np.ascontiguousarray(q_b.reshape(NKB, 128).T)
    hb = h_bias[0, :, 0, 0]                      # [H]
    bias = np.ascontiguousarray(
        np.tile(np.repeat(hb, NQ), 2)[None, :].astype(np.float32))
    ssb = np.ascontiguousarray(
        np.broadcast_to(np.array([s_v, s_q], dtype=np.float32)[None, :],
                        (128, 2)))

    in_maps = []
    for c in range(N_CORES):
        vc = v[c * B_LOC:(c + 1) * B_LOC]        # [B_LOC, NV, DV]
        qc = q[c * B_LOC:(c + 1) * B_LOC]        # [B_LOC, NQ, DQ]
        vt_c = np.ascontiguousarray(
            vc.reshape(B_LOC, NV, TV, 128).transpose(2, 3, 0, 1)
            .reshape(TV // 2, 2, 128, MV).transpose(0, 2, 1, 3)
            .reshape(TV // 2, 128, 2 * MV).astype(bf16))
        qt_c = np.ascontiguousarray(
            qc.reshape(B_LOC, NQ, TQ, 128).transpose(2, 3, 0, 1)
            .reshape(TQ // 4, 4, 128, MQ).transpose(0, 2, 1, 3)
            .reshape(TQ // 4, 128, 4 * MQ).astype(bf16))
        in_maps.append({
            "vt": vt_c, "qt": qt_c, "wv": wv_r, "wq": wq_r,
            "msb": msb, "vb": vb_r, "qb": qb_r, "bias": bias, "ssb": ssb,
        })
    return in_maps


def _run(inputs, trace=False):
    if "nc" not in _CACHE:
        _CACHE["nc"] = _build_program()
    nc = _CACHE["nc"]
    in_maps = _prep_host(inputs)
    res = run_bass_kernel_spmd(nc, in_maps, list(range(N_CORES)), trace=trace)
    out = np.empty((B, H, NV, NQ), dtype=np.float32)
    for c in range(N_CORES):
        oc = res.results[c]["out"]               # [128, B_LOC*H*NQ]
        out[c * B_LOC:(c + 1) * B_LOC] = (
            oc.reshape(NV, B_LOC, H, NQ).transpose(1, 2, 0, 3))
    return out, res


def kernel(**inputs):
    return _run(inputs)[0]


# revision 12
# speedup vs baseline: 1.0570x; 1.0084x over previous
"""Trainium2 Bass kernel for nn_BC_5274219839877.

Computes, for b=64, n_v=128, n_q=32, d_v=2048, d_q=1024, K=3072, H=8:
    v_ = relu((v_g/||v_w||) * v @ v_w^T + v_b)        [b, n_v, K]
    q_ = relu((q_g/||q_w||) * q @ q_w^T + q_b)        [b, n_q, K]
    out[b,h,i,j] = sum_k hm[h,k] v_[b,i,k] q_[b,j,k] + h_bias[h]

Sharding: data-parallel over batch across 8 NeuronCores (8 batches/core),
weights replicated. The whole pipeline is fused and k-blocked on-device;
v_/q_ never touch DRAM.

All matmul operands are f32r (measured faster per row than bf16 on this
part), DMA'd directly into f32r SBUF tiles — no staging casts. The bhvq
contraction over k accumulates directly in PSUM banks across all 24
k-blocks (groups opened by zero matmuls, since start=True zeroes the
whole 2KB bank region), so there are no per-block DVE accumulate adds.
"""

import numpy as np

import concourse.bass as bass
import concourse.tile as tile
from concourse import bacc, mybir
from concourse.bass_utils import run_bass_kernel_spmd

F32 = mybir.dt.float32
F32R = mybir.dt.float32r
BF16 = mybir.dt.bfloat16

N_CORES = 8
B = 64
B_LOC = B // N_CORES       # 8 batches per core
NV = 128
NQ = 32
DV = 2048
DQ = 1024
K = 3072
H = 8

KB = 128                   # k-block size (PSUM partition dim)
NKB = K // KB              # 24 k-blocks
TV = DV // 128             # 16 d-tiles (v side)
TQ = DQ // 128             # 8 d-tiles (q side)
MV = B_LOC * NV            # 1024
MQ = B_LOC * NQ            # 256

N_WARM = 20                # 256-row warm-up matmuls (PE p-state ramp)

_CACHE = {}


def _build_program():
    nc = bacc.Bacc("TRN2", target_bir_lowering=False, debug=False,
                   num_devices=N_CORES)

    vt_d = nc.dram_tensor("vt", [TV // 4, 128, 4 * MV], BF16,
                          kind="ExternalInput")
    qt_d = nc.dram_tensor("qt", [128, TQ * MQ], BF16,
                          kind="ExternalInput")
    wv_d = nc.dram_tensor("wv", [NKB, 128, TV * KB], BF16,
                          kind="ExternalInput")
    wq_d = nc.dram_tensor("wq", [NKB, 128, TQ * KB], BF16,
                          kind="ExternalInput")
    msb_d = nc.dram_tensor("msb", [128, NKB * H], F32, kind="ExternalInput")
    vb_d = nc.dram_tensor("vb", [128, NKB], F32, kind="ExternalInput")
    qb_d = nc.dram_tensor("qb", [128, NKB], F32, kind="ExternalInput")
    bias_d = nc.dram_tensor("bias", [1, 2 * H * NQ], F32R,
                            kind="ExternalInput")
    ssb_d = nc.dram_tensor("ssb", [128, 2], F32, kind="ExternalInput")
    out_d = nc.dram_tensor("out", [128, B_LOC * H * NQ], F32,
                           kind="ExternalOutput")

    relu = mybir.ActivationFunctionType.Relu

    # fixed SBUF allocations (no tile-pool slot recycling)
    msb = nc.alloc_sbuf_tensor("msb_s", [128, NKB * H], F32).ap()
    vb = nc.alloc_sbuf_tensor("vb_s", [128, NKB], F32).ap()
    qb = nc.alloc_sbuf_tensor("qb_s", [128, NKB], F32).ap()
    bias = nc.alloc_sbuf_tensor("bias_s", [1, 2 * H * NQ], F32R).ap()
    ssb = nc.alloc_sbuf_tensor("ssb_s", [128, 2], F32).ap()
    vt_big = nc.alloc_sbuf_tensor("vts", [128, TV * MV], F32R).ap()
    qt_big = nc.alloc_sbuf_tensor("qts", [128, TQ * MQ], F32R).ap()
    wv_g = [nc.alloc_sbuf_tensor(f"wvg{i}", [128, TV * KB], BF16).ap()
            for i in range(2)]
    wq_g = [nc.alloc_sbuf_tensor(f"wqg{i}", [128, TQ * KB], BF16).ap()
            for i in range(2)]
    vt_g = [nc.alloc_sbuf_tensor(f"vtg{i}", [128, 4 * MV], BF16).ap()
            for i in range(2)]
    wv_s = [nc.alloc_sbuf_tensor(f"wvs{i}", [128, TV * KB], F32R).ap()
            for i in range(4)]
    wq_s = [nc.alloc_sbuf_tensor(f"wqs{i}", [128, TQ * KB], F32R).ap()
            for i in range(4)]
    vk = [nc.alloc_sbuf_tensor(f"vk{i}", [128, MV], F32R).ap()
          for i in range(3)]
    qk = [nc.alloc_sbuf_tensor(f"qk{i}", [128, MQ], F32).ap()
          for i in range(2)]
    qx = [nc.alloc_sbuf_tensor(f"qx{i}", [128, H * MQ], F32R).ap()
          for i in range(2)]
    oacc = nc.alloc_sbuf_tensor("oacc", [128, B_LOC * H * NQ], F32).ap()

    ones = nc.alloc_sbuf_tensor("ones", [1, 128], F32).ap()
    warm = nc.alloc_sbuf_tensor("warm", [128, 512], BF16).ap()
    psv = [nc.alloc_psum_tensor(f"psv{i}", [128, 512], F32).ap()
           for i in range(2)]
    psq = [nc.alloc_psum_tensor(f"psq{i}", [128, MQ], F32).ap()
           for i in range(2)]
    psa = [nc.alloc_psum_tensor(f"psa{i}", [128, 512], F32).ap()
           for i in range(4)]

    with tile.TileContext(nc) as tc:
        # --- persistent small loads (gpsimd swdge; big drain overlaps) ---
        nc.gpsimd.dma_start(msb, msb_d.ap())
        nc.gpsimd.dma_start(vb, vb_d.ap())
        nc.gpsimd.dma_start(qb, qb_d.ap())
        nc.gpsimd.dma_start(bias, bias_d.ap())
        nc.gpsimd.dma_start(ssb, ssb_d.ap())

        def issue_weights(kb):
            g = wv_g[kb % 2]
            nc.sync.dma_start(g, wv_d[kb])
            nc.vector.tensor_copy(wv_s[kb % 4], g)
            g2 = wq_g[kb % 2]
            nc.scalar.dma_start(g2, wq_d[kb])
            nc.vector.tensor_copy(wq_s[kb % 4], g2)

        # PE pre-warm: dummy matmuls on zeros while the prologue DMAs
        # stream, so the real matmul stream starts at the warm PE clock
        nc.vector.memset(warm, 0.0)
        for i in range(N_WARM):
            nc.tensor.matmul(psq[0][:], warm[:, :128], warm[:, :MQ],
                             start=True, stop=True)
        # open the four stage-3 PSUM accumulation groups with zeros:
        # start=True zeroes the whole 2KB bank, so each bank gets exactly
        # one start and the per-(kb,b) matmuls below only ever accumulate
        for i in range(4):
            nc.tensor.matmul(psa[i][:], warm[:, :128], warm[:],
                             start=True, stop=False, skip_group_check=True)

        # All big inputs ship as bf16 at half the bytes and are upcast
        # to f32r on-chip. dma_starts carry ~2-3us fixed overhead each
        # (16-engine fanout), so vt goes as four 1MB chunks, qt as one.
        # Casts split across vector (even chunks) and scalar Copy (odd).
        # kb1 weights slot between vt chunks so kb1 never stalls.
        cpf = mybir.ActivationFunctionType.Copy
        nc.vector.memset(ones, 1.0)
        issue_weights(0)

        def vt_chunk(c):
            eng = nc.sync if c % 2 == 0 else nc.scalar
            g = vt_g[c % 2]
            eng.dma_start(g, vt_d[c])
            dst = vt_big[:, c * 4 * MV:(c + 1) * 4 * MV]
            if c % 2 == 0:
                nc.vector.tensor_copy(dst, g)
            else:
                nc.scalar.activation(dst, g, cpf)

        vt_chunk(0)
        vt_chunk(1)
        issue_weights(1)
        vt_chunk(2)
        vt_chunk(3)
        nc.scalar.dma_start(vt_g[0][:, :TQ * MQ], qt_d.ap())
        nc.vector.tensor_copy(qt_big, vt_g[0][:, :TQ * MQ])
        issue_weights(2)

        def stage3(kb):
            # psa[b//2][:, (b%2)*256:] += vk[:, b].T @ Qx[:, b, :, :]
            # accumulated in PSUM across all NKB k-blocks
            vkb = vk[kb % 3]
            qxb = qx[kb % 2]
            for b_ in range(B_LOC):
                nc.tensor.matmul(
                    psa[b_ // 2][:, (b_ % 2) * H * NQ:(b_ % 2 + 1) * H * NQ],
                    vkb[:, b_ * NV:(b_ + 1) * NV],
                    qxb[:, b_ * H * NQ:(b_ + 1) * H * NQ],
                    start=False, stop=False,
                    skip_group_check=True)

        # --- k-blocked fused pipeline ---
        for kb in range(NKB):
            wvb = wv_s[kb % 4]
            wqb = wq_s[kb % 4]

            # stage 1 (v): vk[k, m] = relu(s_v * (v @ v_w^T)^T + v_b)
            vkb = vk[kb % 3]
            for mc in range(MV // 512):
                ps = psv[mc]
                for t in range(TV):
                    nc.tensor.matmul(
                        ps[:],
                        wvb[:, t * KB:(t + 1) * KB],
                        vt_big[:, t * MV + mc * 512:t * MV + (mc + 1) * 512],
                        start=(t == 0), stop=(t == TV - 1))
                nc.scalar.activation(
                    vkb[:, mc * 512:(mc + 1) * 512], ps[:], relu,
                    bias=vb[:, kb:kb + 1], scale=ssb[:, 0:1])

            # stage 1 (q): qk[k, m2]
            qp = psq[kb % 2]
            for t in range(TQ):
                nc.tensor.matmul(
                    qp[:],
                    wqb[:, t * KB:(t + 1) * KB],
                    qt_big[:, t * MQ:(t + 1) * MQ],
                    start=(t == 0), stop=(t == TQ - 1))
            qkb = qk[kb % 2]
            nc.scalar.activation(qkb[:], qp[:], relu,
                                 bias=qb[:, kb:kb + 1], scale=ssb[:, 1:2])

            # stage 2: Qx[k, b*(H*NQ) + h*NQ + j] = hm[h, k] * qk[k, (b,j)]
            qxb = qx[kb % 2]
            qx4 = qxb.rearrange("p (b h j) -> p b h j", b=B_LOC, h=H)
            qk3 = qkb.rearrange("p (b j) -> p b j", b=B_LOC)
            for h in range(H):
                nc.vector.tensor_scalar_mul(
                    qx4[:, :, h, :], qk3[:, :, :],
                    msb[:, kb * H + h:kb * H + h + 1])

            # stage 3, pipelined one k-block behind so its matmuls never
            # head-of-line block ready stage-1 matmuls on the PE stream
            if kb >= 1:
                stage3(kb - 1)

            # prefetch weights for kb+3: 4-deep f32r buffers mean the
            # cast target was last read at kb-1, so no WAR gate; 2-deep
            # bf16 staging is safe because the cast runs within this kb
            if kb + 3 < NKB:
                issue_weights(kb + 3)

        stage3(NKB - 1)

        # epilogue: h_bias folded in exactly via a rank-1 PE accumulate
        # per bank, then PSUM read-out split across vector+scalar, each
        # bank's DMA issued as soon as its copy lands
        cp = mybir.ActivationFunctionType.Copy
        for i in range(4):
            nc.tensor.matmul(psa[i][:], ones.bitcast(F32R)[:], bias[:],
                             start=False, stop=True, skip_group_check=True)
            if i % 2 == 0:
                nc.vector.tensor_copy(oacc[:, i * 512:(i + 1) * 512],
                                      psa[i][:])
            else:
                nc.scalar.activation(oacc[:, i * 512:(i + 1) * 512],
                                     psa[i][:], cp)
            eng = nc.sync if i % 2 == 0 else nc.scalar
            eng.dma_start(out_d.ap()[:, i * 512:(i + 1) * 512],
                          oacc[:, i * 512:(i + 1) * 512])

    nc.compile()
    return nc


def _prep_host(inputs):
    v = np.ascontiguousarray(np.asarray(inputs["v"], dtype=np.float32))
    q = np.ascontiguousarray(np.asarray(inputs["q"], dtype=np.float32))
    v_w = np.asarray(inputs["v_w"], dtype=np.float32)
    q_w = np.asarray(inputs["q_w"], dtype=np.float32)
    v_g = float(np.asarray(inputs["v_g"], dtype=np.float32))
    q_g = float(np.asarray(inputs["q_g"], dtype=np.float32))
    v_b = np.asarray(inputs["v_b"], dtype=np.float32)
    q_b = np.asarray(inputs["q_b"], dtype=np.float32)
    h_mat = np.asarray(inputs["h_mat"], dtype=np.float32)
    h_bias = np.asarray(inputs["h_bias"], dtype=np.float32)

    s_v = v_g / float(np.linalg.norm(v_w.astype(np.float32)))
    s_q = q_g / float(np.linalg.norm(q_w.astype(np.float32)))

    bf16 = mybir.dt.np(BF16)
    wv_r = np.ascontiguousarray(
        v_w.reshape(NKB, KB, TV, 128).transpose(0, 3, 2, 1)
        .reshape(NKB, 128, TV * KB).astype(bf16))
    wq_r = np.ascontiguousarray(
        q_w.reshape(NKB, KB, TQ, 128).transpose(0, 3, 2, 1)
        .reshape(NKB, 128, TQ * KB).astype(bf16))
    hm = h_mat[0, :, 0, :]                       # [H, K]
    msb = np.ascontiguousarray(
        hm.T.reshape(NKB, 128, H).transpose(1, 0, 2).reshape(128, NKB * H))
    vb_r = np.ascontiguousarray(v_b.reshape(NKB, 128).T)
    qb_r = np.ascontiguousarray(q_b.reshape(NKB, 128).T)
    hb = h_bias[0, :, 0, 0]                      # [H]
    bias = np.ascontiguousarray(
        np.tile(np.repeat(hb, NQ), 2)[None, :].astype(np.float32))
    ssb = np.ascontiguousarray(
        np.broadcast_to(np.array([s_v, s_q], dtype=np.float32)[None, :],
                        (128, 2)))

    in_maps = []
    for c in range(N_CORES):
        vc = v[c * B_LOC:(c + 1) * B_LOC]        # [B_LOC, NV, DV]
        qc = q[c * B_LOC:(c + 1) * B_LOC]        # [B_LOC, NQ, DQ]
        vt_c = np.ascontiguousarray(
            vc.reshape(B_LOC, NV, TV, 128).transpose(2, 3, 0, 1)
            .reshape(TV // 4, 4, 128, MV).transpose(0, 2, 1, 3)
            .reshape(TV // 4, 128, 4 * MV).astype(bf16))
        qt_c = np.ascontiguousarray(
            qc.reshape(B_LOC, NQ, TQ, 128).transpose(2, 3, 0, 1)
            .reshape(TQ, 128, MQ).transpose(1, 0, 2)
            .reshape(128, TQ * MQ).astype(bf16))
        in_maps.append({
            "vt": vt_c, "qt": qt_c, "wv": wv_r, "wq": wq_r,
            "msb": msb, "vb": vb_r, "qb": qb_r, "bias": bias, "ssb": ssb,
        })
    return in_maps


def _run(inputs, trace=False):
    if "nc" not in _CACHE:
        _CACHE["nc"] = _build_program()
    nc = _CACHE["nc"]
    in_maps = _prep_host(inputs)
    res = run_bass_kernel_spmd(nc, in_maps, list(range(N_CORES)), trace=trace)
    out = np.empty((B, H, NV, NQ), dtype=np.float32)
    for c in range(N_CORES):
        oc = res.results[c]["out"]               # [128, B_LOC*H*NQ]
        out[c * B_LOC:(c + 1) * B_LOC] = (
            oc.reshape(NV, B_LOC, H, NQ).transpose(1, 2, 0, 3))
    return out, res


def kernel(**inputs):
    return _run(inputs)[0]
